# revision 19
# baseline (speedup 1.0000x reference)
"""LinearCapsPro forward on 8 TRN2 NeuronCores.

Math: out[b,c] = sqrt(u^T sigma u), u = W_c x_b, sigma = (W_c W_c^T + eps I)^-1.
Host-side fold: G_c = W_c W_c^T + eps I = L_c L_c^T  =>  u^T G^-1 u = ||L_c^-1 u||^2.
With W'_c = L_c^-1 W_c the device kernel is just v = x @ W'^T, then
out[b,c] = sqrt(sum_d v[b, c*16+d]^2) - one big matmul + square + group-sum + sqrt.

Sharding: data-parallel over batch (512 rows/core), W' replicated; no collectives.

Inputs ship as fp8e4 (x scaled by SX, W' by SW; the scale comes back out via the
ACT square's input-scale: (v*1/(SX*SW))^2 == v_true^2). fp8 halves the dominant
per-exec cost on this runtime - input staging into the NEFF - and halves HBM
traffic. Rel err vs the f64 reference is ~1e-2 (tolerance 2e-2), dominated by
fp8 quantization of x and W'.

Host also pre-arranges both operands into the exact SBUF layout
(x_sb[p, k, m] = x^T[k*128+p, m]; w_sb[p, s, k, n] = W'^T[k*128+p, s*400+n]) so
every DMA-in is a contiguous per-partition copy - no strided gathers.

Schedule (per core, TimelineSim span ~52.6us vs ~50us practical floor;
PE busy 43.1us ~= the 42.7us fp8 stream minimum):
  - x and w-stripe-0 split at k-edges (0,2,4,8,12,16); the k<2 pieces issue
    first on the SP ring (its queue starts earliest - no activation-table
    load ahead), later pieces alternate ACT/SP rings. Edges tuned by sim
    sweep: each dma_start costs ~625ns of serialized HWDGE descriptor-gen,
    so coarse pieces win once the PE is streaming. w stripes 1-3 are one
    DMA each, prefetching behind compute.
  - Compute loops stripe(4 x 400 cd-cols) -> k(16) -> m(4 x 128 batch rows):
    4 PSUM banks live per stripe (double-buffered across stripes = 8 banks).
  - Epilogue per (stripe, m): ACT square (pre-scaled) psum->sbuf, DVE
    group-sum(16) into a per-m [128,100] result tile; final ACT sqrt + one
    output DMA per m.

A DoubleRow fp8 variant (double_row=True) is correct but measured slower:
this toolchain's legalization emits one 256-col LDWEIGHTS per matmul (no
stationary reuse, FWL off in DR mode), which dominates the N=400 stream.
"""

import sys

import numpy as np

try:
    import concourse  # noqa: F401
except ImportError:  # fresh grading dir: concourse lives in the RL repo
    sys.path.insert(0, "/opt/trn_rl_repo")

B, F, C, D = 4096, 2048, 100, 16
N_CORES = 8
BL = B // N_CORES  # 512 batch rows per core
CD = C * D  # 1600
EPS = 1e-4
KT = F // 128  # 16 contraction tiles
MT = BL // 128  # 4 batch tiles per core
NS = 400  # cd-stripe width (uniform; 4 stripes; 25 capsules each)
ST = CD // NS  # 4 stripes
SX = 16.0  # fp8 pre-scale for x  (|x| < 6  -> |x*SX| < 96, fp8e4 max 240)
SW = 512.0  # fp8 pre-scale for W' (|W'| < 0.05 -> |W'*SW| < 24)

_cached_nc = None


def build_bass(repeat=1, double_row=False):
    """repeat>1 builds a NEFF with the compute body repeated (same output) -
    used only for launch-overhead-immune slope timing, never for grading."""
    import concourse.bacc as bacc
    import concourse.mybir as mybir
    import concourse.tile as tile

    fp8 = mybir.dt.float8e4
    f32 = mybir.dt.float32
    nc = bacc.Bacc("TRN2", target_bir_lowering=False, debug=False, num_devices=N_CORES)
    KT2 = KT // 2  # k-pair tiles for DoubleRow
    if double_row:
        xT = nc.dram_tensor("xT", [128, KT2, 2, BL], fp8, kind="ExternalInput")
        wT = nc.dram_tensor("wT", [128, ST, KT2, 2, NS], fp8, kind="ExternalInput")
    else:
        xT = nc.dram_tensor("xT", [128, KT, BL], fp8, kind="ExternalInput")
        wT = nc.dram_tensor("wT", [128, ST, KT, NS], fp8, kind="ExternalInput")
    out = nc.dram_tensor("out", [BL, C], f32, kind="ExternalOutput")

    with tile.TileContext(nc) as tc:
        with (
            tc.tile_pool(name="xp", bufs=1) as xp,
            tc.tile_pool(name="wp", bufs=1) as wp,
            tc.tile_pool(name="ps", bufs=2, space="PSUM") as psp,
            tc.tile_pool(name="ep", bufs=4) as ep,
            tc.tile_pool(name="rp", bufs=1) as rp,
        ):
            # x on the ACT ring, w on the SP ring. Matching k-piece edges,
            # issued interleaved (x_i, w_i) in compute order: tiny leading
            # pieces so the first matmuls start ~2us in, coarser later pieces
            # to bound per-DMA descriptor-gen overhead.
            nk = KT2 if double_row else KT
            edges = [e for e in (0, 2, 4, 8, 12, 16) if e <= nk]
            if double_row:
                xsb = xp.tile([128, KT2, 2, BL], fp8)
                wsb = wp.tile([128, ST, KT2, 2, NS], fp8)
                nc.sync.dma_start(xsb[:, 0 : edges[1]], xT[:, 0 : edges[1]])
                nc.sync.dma_start(wsb[:, 0, 0 : edges[1]], wT[:, 0, 0 : edges[1]])
                for a, b in zip(edges[1:-1], edges[2:]):
                    nc.scalar.dma_start(xsb[:, a:b], xT[:, a:b])
                    nc.sync.dma_start(wsb[:, 0, a:b], wT[:, 0, a:b])
                for s in range(1, ST):
                    nc.sync.dma_start(wsb[:, s], wT[:, s])

                def xap(k, m):
                    return xsb[:, k, :, m * 128 : (m + 1) * 128]

                def wap(s, k):
                    return wsb[:, s, k, :, :]
            else:
                # separate x / w tiles (a single fused tile makes the PE's
                # operand reads contend with in-flight DMA writes to the same
                # tile - sim'd 5us slower). Leading (k<2) pieces of both go on
                # the SP ring (its queue starts earliest - no activation-table
                # load ahead). Edges tuned by TimelineSim sweep: each dma_start
                # costs ~625ns of serialized HWDGE descriptor-gen, so coarse
                # pieces win once the PE is streaming.
                xsb = xp.tile([128, KT, BL], fp8)
                wsb = wp.tile([128, ST, KT, NS], fp8)
                nc.sync.dma_start(xsb[:, 0 : edges[1]], xT[:, 0 : edges[1]])
                nc.sync.dma_start(wsb[:, 0, 0 : edges[1]], wT[:, 0, 0 : edges[1]])
                for a, b in zip(edges[1:-1], edges[2:]):
                    nc.scalar.dma_start(xsb[:, a:b], xT[:, a:b])
                    nc.sync.dma_start(wsb[:, 0, a:b], wT[:, 0, a:b])
                for s in range(1, ST):
                    nc.sync.dma_start(wsb[:, s], wT[:, s])

                def xap(k, m):
                    return xsb[:, k, m * 128 : (m + 1) * 128]

                def wap(s, k):
                    return wsb[:, s, k, :]
            sq_scale = 1.0 / (SX * SW)
            for r in range(repeat):
                res = [
                    rp.tile([128, C], f32, tag=f"res{m}", name=f"res_r{r}_m{m}")
                    for m in range(MT)
                ]
                for s in range(ST):
                    pss = [
                        psp.tile([128, NS], f32, tag=f"ps{m}", name=f"ps_s{s}_m{m}")
                        for m in range(MT)
                    ]
                    # k-outer on early stripes (matches x/w DMA arrival order);
                    # m-outer on the last stripe so m0-m2's epilogues overlap the
                    # remaining matmuls instead of serializing after them
                    if s < ST - 1:
                        order = [(k, m) for k in range(nk) for m in range(MT)]
                    else:
                        order = [(k, m) for m in range(MT) for k in range(nk)]
                    for k, m in order:
                        nc.tensor.matmul(
                            pss[m][:],
                            xap(k, m),  # lhsT [K, (2,) M]
                            wap(s, k),  # rhs [K, (2,) N]
                            start=(k == 0),
                            stop=(k == nk - 1),
                            perf_mode=(
                                mybir.MatmulPerfMode.DoubleRow if double_row else None
                            ),
                        )
                    ncaps = NS // D  # 25
                    for m in range(MT):
                        sq = ep.tile([128, NS], f32, tag="sq")
                        nc.scalar.activation(
                            sq[:],
                            pss[m][:],
                            mybir.ActivationFunctionType.Square,
                            scale=sq_scale,
                        )
                        nc.vector.reduce_sum(
                            res[m][:, s * ncaps : (s + 1) * ncaps],
                            sq[:].rearrange("p (c d) -> p c d", d=D),
                            axis=mybir.AxisListType.X,
                        )
                # final sqrt + store, split in halves on the (tail-idle) SP
                # ring: the first half's DMA descriptor-gen/queue/transfer
                # overlaps the second half's sqrt, shortening the terminal
                # chain; ACT-ring gens would serialize with the sqrts instead
                for m in range(MT):
                    for c0, c1 in ((0, C // 2), (C // 2, C)):
                        nc.scalar.sqrt(res[m][:, c0:c1], res[m][:, c0:c1])
                        nc.sync.dma_start(
                            out[m * 128 : (m + 1) * 128, c0:c1], res[m][:, c0:c1]
                        )
    nc.compile()
    return nc


def prep_inputs(x: np.ndarray, weight: np.ndarray, double_row=False):
    """Host-side fold + fp8 quantize + SBUF-layout pre-arrange + shard."""
    import ml_dtypes

    fp8 = ml_dtypes.float8_e4m3  # IEEE e4m3 (max 240) == TRN FP8_EXP4
    W64 = weight.astype(np.float64)  # [C, D, F]
    G = np.einsum("cdf,cef->cde", W64, W64)
    G[:, np.arange(D), np.arange(D)] += EPS
    L = np.linalg.cholesky(G)
    Wp = np.linalg.solve(L, W64).reshape(CD, F)  # L^-1 W : [CD, F]
    W8 = np.clip(Wp * SW, -240.0, 240.0).astype(fp8)  # [CD, F]
    X8 = np.clip(x.astype(np.float64) * SX, -240.0, 240.0).astype(fp8)  # [B, F]
    if double_row:
        # w_sb[p, s, k2, j, n] = W'[s*NS+n, k2*256+j*128+p]
        w_sb = np.ascontiguousarray(
            W8.reshape(ST, NS, KT // 2, 2, 128).transpose(4, 0, 2, 3, 1)
        )  # [128, ST, KT2, 2, NS]
    else:
        # w_sb[p, s, k, n] = W'[s*NS+n, k*128+p]
        w_sb = np.ascontiguousarray(
            W8.reshape(ST, NS, KT, 128).transpose(3, 0, 2, 1)
        )  # [128, ST, KT, NS]
    in_maps = []
    for i in range(N_CORES):
        xi = X8[i * BL : (i + 1) * BL]  # [BL, F]
        if double_row:
            # x_sb[p, k2, j, m] = x[m, k2*256+j*128+p]
            x_sb = np.ascontiguousarray(
                xi.reshape(BL, KT // 2, 2, 128).transpose(3, 1, 2, 0)
            )
        else:
            # x_sb[p, k, m] = x[m, k*128+p]
            x_sb = np.ascontiguousarray(xi.reshape(BL, KT, 128).transpose(2, 1, 0))
        in_maps.append({"xT": x_sb, "wT": w_sb})
    return in_maps


def kernel(x: np.ndarray, weight: np.ndarray) -> np.ndarray:
    global _cached_nc
    x = np.asarray(x)
    weight = np.asarray(weight)
    assert x.shape == (B, F) and weight.shape == (C, D, F), (x.shape, weight.shape)
    in_maps = prep_inputs(x, weight)
    if _cached_nc is None:
        _cached_nc = build_bass()
    from concourse.bass_utils import run_bass_kernel_spmd

    res = run_bass_kernel_spmd(_cached_nc, in_maps, core_ids=list(range(N_CORES)))
    return np.concatenate(
        [res.results[i]["out"] for i in range(N_CORES)], axis=0
    ).astype(np.float32)


# revision 20
# speedup vs baseline: 1.0668x; 1.0668x over previous
"""LinearCapsPro forward on 8 TRN2 NeuronCores.

Math: out[b,c] = sqrt(u^T sigma u), u = W_c x_b, sigma = (W_c W_c^T + eps I)^-1.
Host-side fold: G_c = W_c W_c^T + eps I = L_c L_c^T  =>  u^T G^-1 u = ||L_c^-1 u||^2.
With W'_c = L_c^-1 W_c the device kernel is just v = x @ W'^T, then
out[b,c] = sqrt(sum_d v[b, c*16+d]^2) - one big matmul + square + group-sum + sqrt.

Sharding: data-parallel over batch (512 rows/core), W' replicated; no collectives.

Inputs ship as fp8e4 (x scaled by SX, W' by SW; the scale comes back out via the
ACT square's input-scale: (v*1/(SX*SW))^2 == v_true^2). fp8 halves the dominant
per-exec cost on this runtime - input staging into the NEFF - and halves HBM
traffic. Rel err vs the f64 reference is ~1e-2 (tolerance 2e-2), dominated by
fp8 quantization of x and W'.

Host also pre-arranges both operands into the exact SBUF layout
(x_sb[p, k, m] = x^T[k*128+p, m]; w_sb[p, s, k, n] = W'^T[k*128+p, s*400+n]) so
every DMA-in is a contiguous per-partition copy - no strided gathers.

Schedule (per core, TimelineSim span ~52.6us vs ~50us practical floor;
PE busy 43.1us ~= the 42.7us fp8 stream minimum):
  - x and w-stripe-0 split at k-edges (0,2,4,8,12,16); the k<2 pieces issue
    first on the SP ring (its queue starts earliest - no activation-table
    load ahead), later pieces alternate ACT/SP rings. Edges tuned by sim
    sweep: each dma_start costs ~625ns of serialized HWDGE descriptor-gen,
    so coarse pieces win once the PE is streaming. w stripes 1-3 are one
    DMA each, prefetching behind compute.
  - Compute loops stripe(4 x 400 cd-cols) -> k(16) -> m(4 x 128 batch rows):
    4 PSUM banks live per stripe (double-buffered across stripes = 8 banks).
  - Epilogue per (stripe, m): ACT square (pre-scaled) psum->sbuf, DVE
    group-sum(16) into a per-m [128,100] result tile; final ACT sqrt + one
    output DMA per m.

A DoubleRow fp8 variant (double_row=True) is correct but measured slower:
this toolchain's legalization emits one 256-col LDWEIGHTS per matmul (no
stationary reuse, FWL off in DR mode), which dominates the N=400 stream.
"""

import sys

import numpy as np

try:
    import concourse  # noqa: F401
except ImportError:  # fresh grading dir: concourse lives in the RL repo
    sys.path.insert(0, "/opt/trn_rl_repo")

B, F, C, D = 4096, 2048, 100, 16
N_CORES = 8
BL = B // N_CORES  # 512 batch rows per core
CD = C * D  # 1600
EPS = 1e-4
KT = F // 128  # 16 contraction tiles
MT = BL // 128  # 4 batch tiles per core
NS = 400  # cd-stripe width (uniform; 4 stripes; 25 capsules each)
ST = CD // NS  # 4 stripes
SX = 16.0  # fp8 pre-scale for x  (|x| < 6  -> |x*SX| < 96, fp8e4 max 240)
SW = 512.0  # fp8 pre-scale for W' (|W'| < 0.05 -> |W'*SW| < 24)

_cached_nc = None


def build_bass(repeat=1, double_row=False):
    """repeat>1 builds a NEFF with the compute body repeated (same output) -
    used only for launch-overhead-immune slope timing, never for grading."""
    import concourse.bacc as bacc
    import concourse.mybir as mybir
    import concourse.tile as tile

    fp8 = mybir.dt.float8e4
    f32 = mybir.dt.float32
    nc = bacc.Bacc("TRN2", target_bir_lowering=False, debug=False, num_devices=N_CORES)
    KT2 = KT // 2  # k-pair tiles for DoubleRow
    if double_row:
        xT = nc.dram_tensor("xT", [128, KT2, 2, BL], fp8, kind="ExternalInput")
        wT = nc.dram_tensor("wT", [128, ST, KT2, 2, NS], fp8, kind="ExternalInput")
    else:
        xT = nc.dram_tensor("xT", [128, KT, BL], fp8, kind="ExternalInput")
        wT = nc.dram_tensor("wT", [128, ST, KT, NS], fp8, kind="ExternalInput")
    out = nc.dram_tensor("out", [BL, C], f32, kind="ExternalOutput")

    with tile.TileContext(nc) as tc:
        with (
            tc.tile_pool(name="xp", bufs=1) as xp,
            tc.tile_pool(name="wp", bufs=1) as wp,
            tc.tile_pool(name="ps", bufs=2, space="PSUM") as psp,
            tc.tile_pool(name="ep", bufs=4) as ep,
            tc.tile_pool(name="rp", bufs=1) as rp,
        ):
            # x on the ACT ring, w on the SP ring. Matching k-piece edges,
            # issued interleaved (x_i, w_i) in compute order: tiny leading
            # pieces so the first matmuls start ~2us in, coarser later pieces
            # to bound per-DMA descriptor-gen overhead.
            nk = KT2 if double_row else KT
            edges = [e for e in (0, 2, 4, 8, 12, 16) if e <= nk]
            if double_row:
                xsb = xp.tile([128, KT2, 2, BL], fp8)
                wsb = wp.tile([128, ST, KT2, 2, NS], fp8)
                nc.sync.dma_start(xsb[:, 0 : edges[1]], xT[:, 0 : edges[1]])
                nc.sync.dma_start(wsb[:, 0, 0 : edges[1]], wT[:, 0, 0 : edges[1]])
                for a, b in zip(edges[1:-1], edges[2:]):
                    nc.scalar.dma_start(xsb[:, a:b], xT[:, a:b])
                    nc.sync.dma_start(wsb[:, 0, a:b], wT[:, 0, a:b])
                for s in range(1, ST):
                    nc.sync.dma_start(wsb[:, s], wT[:, s])

                def xap(k, m):
                    return xsb[:, k, :, m * 128 : (m + 1) * 128]

                def wap(s, k):
                    return wsb[:, s, k, :, :]
            else:
                # separate x / w tiles (a single fused tile makes the PE's
                # operand reads contend with in-flight DMA writes to the same
                # tile - sim'd 5us slower). x rides the SP HW-DGE ring; w
                # (stripe-0 pieces AND stripes 1-3) rides the GPSIMD SW-DGE
                # ring so the two descriptor-gen paths run in parallel instead
                # of serializing on one HWDGE (~625ns per dma_start). Edges
                # tuned by TimelineSim sweep.
                xsb = xp.tile([128, KT, BL], fp8)
                wsb = wp.tile([128, ST, KT, NS], fp8)
                xedges = [e for e in (0, 1, 2, 4, 8, 12, 16) if e <= nk]
                for a, b in zip(xedges[:-1], xedges[1:]):
                    nc.sync.dma_start(xsb[:, a:b], xT[:, a:b])
                for a, b in zip(edges[:-1], edges[1:]):
                    nc.gpsimd.dma_start(wsb[:, 0, a:b], wT[:, 0, a:b])
                for s in range(1, ST):
                    nc.gpsimd.dma_start(wsb[:, s], wT[:, s])

                def xap(k, m):
                    return xsb[:, k, m * 128 : (m + 1) * 128]

                def wap(s, k):
                    return wsb[:, s, k, :]
            sq_scale = 1.0 / (SX * SW)
            for r in range(repeat):
                res = [
                    rp.tile([128, C], f32, tag=f"res{m}", name=f"res_r{r}_m{m}")
                    for m in range(MT)
                ]
                for s in range(ST):
                    pss = [
                        psp.tile([128, NS], f32, tag=f"ps{m}", name=f"ps_s{s}_m{m}")
                        for m in range(MT)
                    ]
                    # k-outer on early stripes (matches x/w DMA arrival order);
                    # m-outer on the last stripe so m0-m2's epilogues overlap the
                    # remaining matmuls instead of serializing after them
                    if s < ST - 1:
                        order = [(k, m) for k in range(nk) for m in range(MT)]
                    else:
                        order = [(k, m) for m in range(MT) for k in range(nk)]
                    for k, m in order:
                        nc.tensor.matmul(
                            pss[m][:],
                            xap(k, m),  # lhsT [K, (2,) M]
                            wap(s, k),  # rhs [K, (2,) N]
                            start=(k == 0),
                            stop=(k == nk - 1),
                            perf_mode=(
                                mybir.MatmulPerfMode.DoubleRow if double_row else None
                            ),
                        )
                    ncaps = NS // D  # 25
                    for m in range(MT):
                        sq = ep.tile([128, NS], f32, tag="sq")
                        nc.scalar.activation(
                            sq[:],
                            pss[m][:],
                            mybir.ActivationFunctionType.Square,
                            scale=sq_scale,
                        )
                        nc.vector.reduce_sum(
                            res[m][:, s * ncaps : (s + 1) * ncaps],
                            sq[:].rearrange("p (c d) -> p c d", d=D),
                            axis=mybir.AxisListType.X,
                        )
                # final sqrt + store, split in halves on the (tail-idle) SP
                # ring: the first half's DMA descriptor-gen/queue/transfer
                # overlaps the second half's sqrt, shortening the terminal
                # chain; ACT-ring gens would serialize with the sqrts instead
                for m in range(MT):
                    for c0, c1 in ((0, C // 2), (C // 2, C)):
                        nc.scalar.sqrt(res[m][:, c0:c1], res[m][:, c0:c1])
                        nc.sync.dma_start(
                            out[m * 128 : (m + 1) * 128, c0:c1], res[m][:, c0:c1]
                        )
    nc.compile()
    return nc


def prep_inputs(x: np.ndarray, weight: np.ndarray, double_row=False):
    """Host-side fold + fp8 quantize + SBUF-layout pre-arrange + shard."""
    import ml_dtypes

    fp8 = ml_dtypes.float8_e4m3  # IEEE e4m3 (max 240) == TRN FP8_EXP4
    W64 = weight.astype(np.float64)  # [C, D, F]
    G = np.einsum("cdf,cef->cde", W64, W64)
    G[:, np.arange(D), np.arange(D)] += EPS
    L = np.linalg.cholesky(G)
    Wp = np.linalg.solve(L, W64).reshape(CD, F)  # L^-1 W : [CD, F]
    W8 = np.clip(Wp * SW, -240.0, 240.0).astype(fp8)  # [CD, F]
    X8 = np.clip(x.astype(np.float64) * SX, -240.0, 240.0).astype(fp8)  # [B, F]
    if double_row:
        # w_sb[p, s, k2, j, n] = W'[s*NS+n, k2*256+j*128+p]
        w_sb = np.ascontiguousarray(
            W8.reshape(ST, NS, KT // 2, 2, 128).transpose(4, 0, 2, 3, 1)
        )  # [128, ST, KT2, 2, NS]
    else:
        # w_sb[p, s, k, n] = W'[s*NS+n, k*128+p]
        w_sb = np.ascontiguousarray(
            W8.reshape(ST, NS, KT, 128).transpose(3, 0, 2, 1)
        )  # [128, ST, KT, NS]
    in_maps = []
    for i in range(N_CORES):
        xi = X8[i * BL : (i + 1) * BL]  # [BL, F]
        if double_row:
            # x_sb[p, k2, j, m] = x[m, k2*256+j*128+p]
            x_sb = np.ascontiguousarray(
                xi.reshape(BL, KT // 2, 2, 128).transpose(3, 1, 2, 0)
            )
        else:
            # x_sb[p, k, m] = x[m, k*128+p]
            x_sb = np.ascontiguousarray(xi.reshape(BL, KT, 128).transpose(2, 1, 0))
        in_maps.append({"xT": x_sb, "wT": w_sb})
    return in_maps


def kernel(x: np.ndarray, weight: np.ndarray) -> np.ndarray:
    global _cached_nc
    x = np.asarray(x)
    weight = np.asarray(weight)
    assert x.shape == (B, F) and weight.shape == (C, D, F), (x.shape, weight.shape)
    in_maps = prep_inputs(x, weight)
    if _cached_nc is None:
        _cached_nc = build_bass()
    from concourse.bass_utils import run_bass_kernel_spmd

    res = run_bass_kernel_spmd(_cached_nc, in_maps, core_ids=list(range(N_CORES)))
    return np.concatenate(
        [res.results[i]["out"] for i in range(N_CORES)], axis=0
    ).astype(np.float32)
